# revision 18
# baseline (speedup 1.0000x reference)
"""CrystalTransformer (TransformerConv x3 + segment-mean pool) on 8 trn2 cores.

Host: sort edges by dst, shard nodes into 8 contiguous 2560-node ranges
(128-aligned, zero-padded to 20480), pad per-dst-block edge lists to a uniform
tile count tpb so all 8 cores run one SPMD program.

Device per core/layer/block (128 dst nodes):
  B1: C = [M_h @ hT_aug]_h  (one [115, 512] matrix per block; M_h =
      w2k_h @ Wq_aug_h^T / sqrt(D) is a host-folded layer constant, so q is
      never materialized), skip = hT_aug^T @ Wskip_aug into the out PSUM.
  B2 per 128-edge tile: gather h[src] (indirect DMA) into X=[h_src|ea|1],
      XT = transpose(X), alphaT = XT^T @ C ([e, h*128+dst], all heads, one
      matmul), EXM = exp(alphaT) * S (S = per-edge one-hot of dst_rel,
      broadcast over heads), aggT += X^T @ EXM ([115, 512], one matmul).
  B3: den = aggT row 114 (ones-column trick), Zn = aggT * (1/den) via
      partition-broadcast, out += [Zn_h^T @ wv2_h]_h, h = relu(out).
AllGather h between layers; pooling via one-hot matmul on batch ids; final
tiny matmul on host. Edge slab ships as bf16 (halves upload), compute f32.
"""
import json
import numpy as np

P = 128
N, E, G = 20000, 320000, 256
DA, DE, D, H, L = 92, 50, 64, 4, 3
NCORES = 8
NLOC = 2560            # node slots per core (20 blocks of 128)
NB = NLOC // P         # 20 dst blocks per core
NPAD = NLOC * NCORES   # 20480
XW = D + DE + 1        # 115 = [h_src(64) | 1 | ea(50)]; ones at aligned row 64
SW = DE + 1            # 51 slab cols per tile: [1 | ea(50)]
HD = H * P             # 512 = heads * dst concatenated


# ---------------------------------------------------------------- BIR patch --
def _install_birpatch():
    """This container's walrus rejects >1 sem wait per instruction; hoist
    extras onto injected preceding Drains (same engine => same order)."""
    import concourse.bass2jax as b2j
    if getattr(b2j, "_birpatch_installed", False):
        return
    orig = b2j.compile_bir_kernel

    def patch(bir_bytes):
        d = json.loads(bir_bytes)
        for fn in d.get("functions", []):
            for blk in fn.get("blocks", []):
                out = []
                for ins in blk.get("instructions", []):
                    si = ins.get("sync_info") or {}
                    waits = si.get("on_wait") or []
                    if len(waits) > 1:
                        for k, w in enumerate(waits[:-1]):
                            out.append({
                                "debug": ins.get("debug", 0),
                                "engine": ins["engine"], "ins": [], "outs": [],
                                "name": f'{ins["name"]}-w{k}', "opcode": "Drain",
                                "sync_info": {"on_update": [], "on_wait": [w]},
                            })
                        si["on_wait"] = waits[-1:]
                    out.append(ins)
                blk["instructions"] = out
        return json.dumps(d).encode()

    def wrapper(bir_str, *a, **kw):
        try:
            bir_str = patch(bir_str)
        except Exception as e:  # pragma: no cover
            print("[birpatch] failed:", e)
        return orig(bir_str, *a, **kw)

    b2j.compile_bir_kernel = wrapper
    b2j._birpatch_installed = True


# ------------------------------------------------------------------- device --
def _build_nc(tpb):
    import concourse.bass as bass
    import concourse.mybir as mybir
    import concourse.tile as tile
    from concourse.masks import make_identity

    f32, i32, bf16 = mybir.dt.float32, mybir.dt.int32, mybir.dt.bfloat16
    Alu, Act = mybir.AluOpType, mybir.ActivationFunctionType
    NT = NB * tpb          # edge tiles per core

    nc = bass.Bass("TRN2", target_bir_lowering=False, debug=False,
                   num_devices=NCORES)
    di = lambda nm, sh, dt=f32: nc.dram_tensor(nm, sh, dt, kind="ExternalInput")
    xaug_in = di("xaugT", [DA + 1, NLOC])
    eas_in = di("ea_slab", [P, NT * SW], bf16)   # [ea | 1] per tile
    met_in = di("met_slab", [P, NT])             # dst_rel per tile (f32)
    idx_in = di("idx_slab", [P, NT], i32)        # src_global per tile
    brel_in = di("batch_rel", [NLOC, 1])
    watom_in = di("w_atom_aug", [DA + 1, D])
    mt_in = di("mt", [L, D + 1, H * XW])         # [Wq_aug_h @ w2k_h^T / 8]_h
    wv2_in = di("wv2", [L, XW, H * D])
    wsk_in = di("wska", [L, D + 1, D])
    out_pool = nc.dram_tensor("out_pool", [P, D + 1], f32, kind="ExternalOutput")

    h_mine = nc.dram_tensor("h_mine", [NLOC, D], f32)
    h_full = [nc.dram_tensor(f"h_full_{l}", [NPAD, D], f32, addr_space="Shared")
              for l in range(L)]

    with tile.TileContext(nc, num_cores=NCORES) as tc:
        import contextlib
        with contextlib.ExitStack() as st:
            cp = st.enter_context(tc.tile_pool(name="const", bufs=1))
            xp = st.enter_context(tc.tile_pool(name="xt", bufs=3))
            vp = st.enter_context(tc.tile_pool(name="dve", bufs=3))
            ps_t = st.enter_context(tc.tile_pool(name="ps_t", bufs=1, space="PSUM"))
            ps_a = st.enter_context(tc.tile_pool(name="ps_a", bufs=2, space="PSUM"))
            ps_g = st.enter_context(tc.tile_pool(name="ps_g", bufs=2, space="PSUM"))
            ps_c = st.enter_context(tc.tile_pool(name="ps_c", bufs=1, space="PSUM"))
            ps_b = st.enter_context(tc.tile_pool(name="ps_b", bufs=1, space="PSUM"))

            ident = cp.tile([P, P], f32)
            make_identity(nc, ident[:])
            iota_i = cp.tile([P, P], i32)
            nc.gpsimd.iota(iota_i[:], pattern=[[1, P]], base=0, channel_multiplier=0)
            iota_f = cp.tile([P, P], f32)
            nc.vector.tensor_copy(iota_f[:], iota_i[:])
            ones_col = cp.tile([P, 1], f32)
            nc.vector.memset(ones_col[:], 1.0)
            ones_row = cp.tile([1, XW], f32)
            nc.vector.memset(ones_row[:], 1.0)
            h_loc = cp.tile([P, NB * D], f32)
            watom_sb = cp.tile([DA + 1, D], f32)
            nc.sync.dma_start(out=watom_sb[:], in_=watom_in[:])
            eas = cp.tile([P, NT * SW], bf16)
            nc.sync.dma_start(out=eas[:], in_=eas_in[:])
            met = cp.tile([P, NT], f32)
            nc.sync.dma_start(out=met[:], in_=met_in[:])
            idxs = cp.tile([P, NT], i32)
            nc.sync.dma_start(out=idxs[:], in_=idx_in[:])

            # ---- embed: h0 = x@W_atom + b (no relu, as in reference)
            for b in range(NB):
                xT = xp.tile([DA + 1, P], f32, tag="hta")
                nc.sync.dma_start(out=xT[:], in_=xaug_in[:, b * P:(b + 1) * P])
                hb_ps = ps_b.tile([P, D], f32, tag="out")
                nc.tensor.matmul(hb_ps[:], lhsT=xT[:], rhs=watom_sb[:],
                                 start=True, stop=True)
                nc.vector.tensor_copy(h_loc[:, b * D:(b + 1) * D], hb_ps[:])
                nc.sync.dma_start(out=h_mine[b * P:(b + 1) * P, :],
                                  in_=h_loc[:, b * D:(b + 1) * D])
            tc.strict_bb_all_engine_barrier()
            nc.gpsimd.collective_compute(
                "AllGather", Alu.bypass,
                replica_groups=[list(range(NCORES))],
                ins=[h_mine.ap().opt()], outs=[h_full[0].ap().opt()])
            tc.strict_bb_all_engine_barrier()

            for l in range(L):
                mt_sb = cp.tile([D + 1, H * XW], f32, tag="mt")
                nc.sync.dma_start(out=mt_sb[:], in_=mt_in[l])
                wv2_sb = cp.tile([XW, H * D], f32, tag="wv2")
                nc.sync.dma_start(out=wv2_sb[:], in_=wv2_in[l])
                wsk_sb = cp.tile([D + 1, D], f32, tag="wsk")
                nc.sync.dma_start(out=wsk_sb[:], in_=wsk_in[l])

                for b in range(NB):
                    # ---- B1: per-block C matrix + skip into out PSUM
                    hT_ps = ps_t.tile([D, P], f32, tag="tr")
                    nc.tensor.transpose(out=hT_ps[:], in_=h_loc[:, b * D:(b + 1) * D],
                                        identity=ident[:])
                    hTa = xp.tile([D + 1, P], f32, tag="hta")
                    nc.vector.memset(hTa[:], 1.0)
                    nc.vector.tensor_copy(hTa[:D, :], hT_ps[:])
                    C_ps = ps_c.tile([XW, HD], f32, tag="C")
                    for h in range(H):
                        nc.tensor.matmul(C_ps[:, h * P:(h + 1) * P],
                                         lhsT=mt_sb[:, h * XW:(h + 1) * XW],
                                         rhs=hTa[:], start=True, stop=True,
                                         skip_group_check=(h > 0))
                    C_sb = vp.tile([XW, HD], f32, tag="C")
                    nc.vector.tensor_copy(C_sb[:], C_ps[:])
                    out_ps = ps_b.tile([P, D], f32, tag="out")
                    nc.tensor.matmul(out_ps[:], lhsT=hTa[:], rhs=wsk_sb[:],
                                     start=True, stop=False)

                    # ---- B2: edge tiles
                    agg_ps = ps_g.tile([XW, HD], f32, tag="agg")
                    for t in range(tpb):
                        g = (b * tpb + t) * SW
                        X = xp.tile([P, XW], f32, tag="X")
                        nc.gpsimd.indirect_dma_start(
                            out=X[:, :D], out_offset=None,
                            in_=h_full[l][:],
                            in_offset=bass.IndirectOffsetOnAxis(
                                ap=idxs[:, b * tpb + t:b * tpb + t + 1], axis=0))
                        nc.vector.tensor_copy(X[:, D:], eas[:, g:g + SW])
                        XT_ps = ps_t.tile([XW, P], f32, tag="tr")
                        nc.tensor.transpose(out=XT_ps[:], in_=X[:], identity=ident[:])
                        XT = xp.tile([XW, P], f32, tag="XT")
                        nc.vector.tensor_copy(XT[:], XT_ps[:])
                        al_ps = ps_a.tile([P, HD], f32, tag="al")
                        nc.tensor.matmul(al_ps[:], lhsT=XT[:], rhs=C_sb[:],
                                         start=True, stop=True)
                        S = vp.tile([P, P], f32, tag="S")
                        nc.gpsimd.tensor_scalar(out=S[:], in0=iota_f[:],
                                                scalar1=met[:, b * tpb + t:
                                                            b * tpb + t + 1],
                                                scalar2=None, op0=Alu.is_equal)
                        EX = vp.tile([P, HD], f32, tag="EX")
                        nc.scalar.activation(EX[:], al_ps[:], Act.Exp)
                        EXM = vp.tile([P, HD], f32, tag="EXM")
                        nc.vector.tensor_tensor(
                            out=EXM[:].rearrange("p (h d) -> p h d", h=H),
                            in0=EX[:].rearrange("p (h d) -> p h d", h=H),
                            in1=S[:, None, :].broadcast_to([P, H, P]),
                            op=Alu.mult)
                        nc.tensor.matmul(agg_ps[:], lhsT=X[:], rhs=EXM[:],
                                         start=(t == 0), stop=(t == tpb - 1))

                    # ---- B3: normalize, project, skip+relu
                    den = vp.tile([1, HD], f32, tag="den")
                    nc.vector.tensor_scalar_max(out=den[:], in0=agg_ps[D:D + 1, :],
                                                scalar1=1e-30)
                    rden = vp.tile([1, HD], f32, tag="rd")
                    nc.vector.reciprocal(rden[:], den[:])
                    rf_ps = ps_c.tile([XW, HD], f32, tag="C")
                    nc.tensor.matmul(rf_ps[:], lhsT=ones_row[:], rhs=rden[:],
                                     start=True, stop=True)
                    rfull = vp.tile([XW, HD], f32, tag="rf")
                    nc.vector.tensor_copy(rfull[:], rf_ps[:])
                    Zn = vp.tile([XW, HD], f32, tag="Zn")
                    nc.vector.tensor_tensor(out=Zn[:], in0=agg_ps[:], in1=rfull[:],
                                            op=Alu.mult)
                    for h in range(H):
                        nc.tensor.matmul(out_ps[:], lhsT=Zn[:, h * P:(h + 1) * P],
                                         rhs=wv2_sb[:, h * D:(h + 1) * D],
                                         start=False, stop=(h == H - 1))
                    nc.vector.tensor_scalar_max(
                        out=h_loc[:, b * D:(b + 1) * D], in0=out_ps[:], scalar1=0.0)
                    if l < L - 1:
                        nc.sync.dma_start(out=h_mine[b * P:(b + 1) * P, :],
                                          in_=h_loc[:, b * D:(b + 1) * D])
                if l < L - 1:
                    tc.strict_bb_all_engine_barrier()
                    nc.gpsimd.collective_compute(
                        "AllGather", Alu.bypass,
                        replica_groups=[list(range(NCORES))],
                        ins=[h_mine.ap().opt()], outs=[h_full[l + 1].ap().opt()])
                    tc.strict_bb_all_engine_barrier()

            # ---- pooling: one-hot on batch ids
            brel = cp.tile([P, NB], f32)
            nc.sync.dma_start(out=brel[:],
                              in_=brel_in[:].rearrange("(b p) o -> p (b o)", p=P))
            pool_ps = ps_a.tile([P, D], f32, tag="al")
            cnt_ps = ps_b.tile([P, 1], f32, tag="cnt")
            for b in range(NB):
                Sb = vp.tile([P, P], f32, tag="S")
                nc.vector.tensor_scalar(out=Sb[:], in0=iota_f[:],
                                        scalar1=brel[:, b:b + 1], scalar2=None,
                                        op0=Alu.is_equal)
                nc.tensor.matmul(pool_ps[:], lhsT=Sb[:],
                                 rhs=h_loc[:, b * D:(b + 1) * D],
                                 start=(b == 0), stop=(b == NB - 1))
                nc.tensor.matmul(cnt_ps[:], lhsT=Sb[:], rhs=ones_col[:],
                                 start=(b == 0), stop=(b == NB - 1),
                                 skip_group_check=True)
            pool_sb = vp.tile([P, D + 1], f32, tag="pool_sb")
            nc.vector.tensor_copy(pool_sb[:, :D], pool_ps[:])
            nc.vector.tensor_copy(pool_sb[:, D:], cnt_ps[:])
            nc.sync.dma_start(out=out_pool[:], in_=pool_sb[:])
    return nc


# --------------------------------------------------------------------- host --
def kernel(**inputs):
    _install_birpatch()
    from concourse.bass_utils import run_bass_kernel_spmd
    import ml_dtypes

    x = np.asarray(inputs["x"], np.float32)
    ei = np.asarray(inputs["edge_index"]).astype(np.int64)
    ea = np.asarray(inputs["edge_attr"], np.float32)
    batch = np.asarray(inputs["batch"]).astype(np.int64)
    Wq = np.asarray(inputs["Wq"], np.float32); bq = np.asarray(inputs["bq"], np.float32)
    Wk = np.asarray(inputs["Wk"], np.float32); bk = np.asarray(inputs["bk"], np.float32)
    Wv = np.asarray(inputs["Wv"], np.float32); bv = np.asarray(inputs["bv"], np.float32)
    We = np.asarray(inputs["We"], np.float32)
    Wskip = np.asarray(inputs["Wskip"], np.float32)
    bskip = np.asarray(inputs["bskip"], np.float32)
    W_atom = np.asarray(inputs["W_atom"], np.float32)
    b_atom = np.asarray(inputs["b_atom"], np.float32)
    W_edge = np.asarray(inputs["W_edge"], np.float32)
    b_edge = np.asarray(inputs["b_edge"], np.float32)
    W_out = np.asarray(inputs["W_out"], np.float32)
    b_out = np.asarray(inputs["b_out"], np.float32)

    src, dst = ei[0], ei[1]
    order = np.argsort(dst, kind="stable")
    src_s, dst_s = src[order], dst[order]
    ea_s = ea[order]

    # per-(core, block) edge ranges; uniform tile count tpb across all
    blk_of = dst_s // P                       # 0..156 (20 blocks x 8 cores)
    nblk = NCORES * NB
    counts = np.bincount(blk_of, minlength=nblk)
    starts = np.zeros(nblk + 1, np.int64)
    np.cumsum(counts, out=starts[1:])
    tpb = int(np.ceil(max(1, counts.max()) / P))
    NT = NB * tpb

    # folds: w2k rows = [Wk ; ones-row (bk + edge-bias) ; Wea@We], per layer
    Wea = np.concatenate([W_edge, b_edge[None, :]], 0)        # [51, 64]
    mt = np.zeros((L, D + 1, H * XW), np.float32)
    wv2 = np.zeros((L, H, XW, D), np.float32)
    wska = np.zeros((L, D + 1, D), np.float32)
    scale = 1.0 / np.sqrt(D)
    for l in range(L):
        ew = Wea @ We[l]                                      # [51, 256]
        w2k = np.zeros((XW, H * D), np.float32)
        w2k[:D] = Wk[l]
        w2k[D] = ew[DE] + bk[l]
        w2k[D + 1:] = ew[:DE]
        Wq_aug = np.concatenate([Wq[l], bq[l][None, :]], 0)   # [65, 256]
        for h in range(H):
            mt[l, :, h * XW:(h + 1) * XW] = (
                Wq_aug[:, h * D:(h + 1) * D] @ w2k[:, h * D:(h + 1) * D].T) * scale
            wv2[l, h, :D] = Wv[l][:, h * D:(h + 1) * D] / H
            wv2[l, h, D] = (ew[DE, h * D:(h + 1) * D]
                            + bv[l][h * D:(h + 1) * D]) / H
            wv2[l, h, D + 1:] = ew[:DE, h * D:(h + 1) * D] / H
        wska[l, :D] = Wskip[l]
        wska[l, D] = bskip[l]
    watom = np.concatenate([W_atom, b_atom[None, :]], 0)
    wv2 = np.ascontiguousarray(np.transpose(wv2, (0, 2, 1, 3)).reshape(L, XW, H * D))

    in_maps, g0s = [], []
    for c in range(NCORES):
        n0 = c * NLOC
        real = min(NLOC, max(0, N - n0))
        xaugT = np.zeros((DA + 1, NLOC), np.float32)
        xaugT[DA] = 1.0
        xaugT[:DA, :real] = x[n0:n0 + real].T
        eslab = np.zeros((P, NB, tpb, SW), np.float32)
        eslab[:, :, :, 0] = 1.0
        mslab = np.full((P, NB, tpb), -1.0, np.float32)
        islab = np.zeros((P, NB, tpb), np.int32)
        for b in range(NB):
            gb = c * NB + b
            s, e = starts[gb], starts[gb + 1]
            k = e - s
            if k == 0:
                continue
            j = np.arange(k)
            tt, pp = j // P, j % P
            mslab[pp, b, tt] = dst_s[s:e] - (n0 + b * P)
            eslab[pp, b, tt, 1:] = ea_s[s:e]
            islab[pp, b, tt] = src_s[s:e]
        brel = np.full((NLOC, 1), -1.0, np.float32)
        g0 = int(batch[min(n0, N - 1)]) if n0 < N else 0
        if real > 0:
            brel[:real, 0] = batch[n0:n0 + real] - g0
        g0s.append(g0)
        in_maps.append({
            "xaugT": xaugT,
            "ea_slab": eslab.reshape(P, NT * SW).astype(ml_dtypes.bfloat16),
            "met_slab": mslab.reshape(P, NT),
            "idx_slab": islab.reshape(P, NT),
            "batch_rel": brel,
            "w_atom_aug": watom, "mt": mt, "wv2": wv2, "wska": wska,
        })

    nc = _build_nc(tpb)
    res = run_bass_kernel_spmd(nc, in_maps, core_ids=list(range(NCORES)))

    sums = np.zeros((G + P, D), np.float64)
    cnts = np.zeros(G + P, np.float64)
    for c in range(NCORES):
        op = res.results[c]["out_pool"]
        sums[g0s[c]:g0s[c] + P] += op[:, :D]
        cnts[g0s[c]:g0s[c] + P] += op[:, D]
    pooled = sums[:G] / np.maximum(cnts[:G], 1.0)[:, None]
    out = pooled.astype(np.float32) @ W_out + b_out
    return out.squeeze()


# revision 19
# speedup vs baseline: 23.8048x; 23.8048x over previous
"""CrystalTransformer (TransformerConv x3 + segment-mean pool) on 8 trn2 cores.

Host: sort edges by dst, shard nodes into 8 contiguous 2560-node ranges
(128-aligned, zero-padded to 20480), pad per-dst-block edge lists to a uniform
tile count tpb so all 8 cores run one SPMD program.

Device per core/layer/block (128 dst nodes):
  B1: C = [M_h @ hT_aug]_h  (one [115, 512] matrix per block; M_h =
      w2k_h @ Wq_aug_h^T / sqrt(D) is a host-folded layer constant, so q is
      never materialized), skip = hT_aug^T @ Wskip_aug into the out PSUM.
  B2 per 128-edge tile: gather h[src] (indirect DMA) into X=[h_src|ea|1],
      XT = transpose(X), alphaT = XT^T @ C ([e, h*128+dst], all heads, one
      matmul), EXM = exp(alphaT) * S (S = per-edge one-hot of dst_rel,
      broadcast over heads), aggT += X^T @ EXM ([115, 512], one matmul).
  B3: den = aggT row 114 (ones-column trick), Zn = aggT * (1/den) via
      partition-broadcast, out += [Zn_h^T @ wv2_h]_h, h = relu(out).
AllGather h between layers; pooling via one-hot matmul on batch ids; final
tiny matmul on host. Edge slab ships as bf16 (halves upload), compute f32.
"""
import json
import numpy as np

P = 128
N, E, G = 20000, 320000, 256
DA, DE, D, H, L = 92, 50, 64, 4, 3
NCORES = 8
NLOC = 2560            # node slots per core (20 blocks of 128)
NB = NLOC // P         # 20 dst blocks per core
NPAD = NLOC * NCORES   # 20480
XW = D + DE + 1        # 115 = [h_src(64) | 1 | ea(50)]; ones at aligned row 64
SW = DE + 1            # 51 slab cols per tile: [1 | ea(50)]
HD = H * P             # 512 = heads * dst concatenated


# ---------------------------------------------------------------- BIR patch --
def _install_birpatch():
    """This container's walrus rejects >1 sem wait per instruction; hoist
    extras onto injected preceding Drains (same engine => same order)."""
    import concourse.bass2jax as b2j
    if getattr(b2j, "_birpatch_installed", False):
        return
    orig = b2j.compile_bir_kernel

    def patch(bir_bytes):
        d = json.loads(bir_bytes)
        for fn in d.get("functions", []):
            for blk in fn.get("blocks", []):
                out = []
                for ins in blk.get("instructions", []):
                    si = ins.get("sync_info") or {}
                    waits = si.get("on_wait") or []
                    if len(waits) > 1:
                        for k, w in enumerate(waits[:-1]):
                            out.append({
                                "debug": ins.get("debug", 0),
                                "engine": ins["engine"], "ins": [], "outs": [],
                                "name": f'{ins["name"]}-w{k}', "opcode": "Drain",
                                "sync_info": {"on_update": [], "on_wait": [w]},
                            })
                        si["on_wait"] = waits[-1:]
                    out.append(ins)
                blk["instructions"] = out
        return json.dumps(d).encode()

    def wrapper(bir_str, *a, **kw):
        try:
            bir_str = patch(bir_str)
        except Exception as e:  # pragma: no cover
            print("[birpatch] failed:", e)
        return orig(bir_str, *a, **kw)

    b2j.compile_bir_kernel = wrapper
    b2j._birpatch_installed = True


# ------------------------------------------------------------------- device --
def _build_nc(tpb):
    import concourse.bass as bass
    import concourse.mybir as mybir
    import concourse.tile as tile
    from concourse.masks import make_identity

    f32, i32, bf16 = mybir.dt.float32, mybir.dt.int32, mybir.dt.bfloat16
    Alu, Act = mybir.AluOpType, mybir.ActivationFunctionType
    NT = NB * tpb          # edge tiles per core

    nc = bass.Bass("TRN2", target_bir_lowering=False, debug=False,
                   num_devices=NCORES)
    di = lambda nm, sh, dt=f32: nc.dram_tensor(nm, sh, dt, kind="ExternalInput")
    xaug_in = di("xaugT", [DA + 1, NLOC])
    eas_in = di("ea_slab", [P, NT * SW], bf16)   # [ea | 1] per tile
    met_in = di("met_slab", [P, NT])             # dst_rel per tile (f32)
    idx_in = di("idx_slab", [P, NT], i32)        # src_global per tile
    brel_in = di("batch_rel", [NLOC, 1])
    watom_in = di("w_atom_aug", [DA + 1, D])
    mt_in = di("mt", [L, D + 1, H * XW])         # [Wq_aug_h @ w2k_h^T / 8]_h
    wv2_in = di("wv2", [L, XW, H * D])
    wsk_in = di("wska", [L, D + 1, D])
    out_pool = nc.dram_tensor("out_pool", [P, D + 1], f32, kind="ExternalOutput")

    h_mine = nc.dram_tensor("h_mine", [NLOC, D], f32)
    h_full = [nc.dram_tensor(f"h_full_{l}", [NPAD, D], f32, addr_space="Shared")
              for l in range(L)]

    with tile.TileContext(nc, num_cores=NCORES) as tc:
        import contextlib
        with contextlib.ExitStack() as st:
            cp = st.enter_context(tc.tile_pool(name="const", bufs=1))
            xp = st.enter_context(tc.tile_pool(name="xt", bufs=3))
            vp = st.enter_context(tc.tile_pool(name="dve", bufs=3))
            ps_t = st.enter_context(tc.tile_pool(name="ps_t", bufs=1, space="PSUM"))
            ps_a = st.enter_context(tc.tile_pool(name="ps_a", bufs=2, space="PSUM"))
            ps_g = st.enter_context(tc.tile_pool(name="ps_g", bufs=2, space="PSUM"))
            ps_c = st.enter_context(tc.tile_pool(name="ps_c", bufs=1, space="PSUM"))
            ps_b = st.enter_context(tc.tile_pool(name="ps_b", bufs=1, space="PSUM"))

            ident = cp.tile([P, P], f32)
            make_identity(nc, ident[:])
            iota_i = cp.tile([P, P], i32)
            nc.gpsimd.iota(iota_i[:], pattern=[[1, P]], base=0, channel_multiplier=0)
            iota_f = cp.tile([P, P], f32)
            nc.vector.tensor_copy(iota_f[:], iota_i[:])
            ones_col = cp.tile([P, 1], f32)
            nc.vector.memset(ones_col[:], 1.0)
            ones_row = cp.tile([1, XW], f32)
            nc.vector.memset(ones_row[:], 1.0)
            h_loc = cp.tile([P, NB * D], f32)
            watom_sb = cp.tile([DA + 1, D], f32)
            nc.sync.dma_start(out=watom_sb[:], in_=watom_in[:])
            eas = cp.tile([P, NT * SW], bf16)
            nc.sync.dma_start(out=eas[:], in_=eas_in[:])
            met = cp.tile([P, NT], f32)
            nc.sync.dma_start(out=met[:], in_=met_in[:])
            idxs = cp.tile([P, NT], i32)
            nc.sync.dma_start(out=idxs[:], in_=idx_in[:])

            # ---- embed: h0 = x@W_atom + b (no relu, as in reference)
            for b in range(NB):
                xT = xp.tile([DA + 1, P], f32, tag="hta")
                nc.sync.dma_start(out=xT[:], in_=xaug_in[:, b * P:(b + 1) * P])
                hb_ps = ps_b.tile([P, D], f32, tag="out")
                nc.tensor.matmul(hb_ps[:], lhsT=xT[:], rhs=watom_sb[:],
                                 start=True, stop=True)
                nc.vector.tensor_copy(h_loc[:, b * D:(b + 1) * D], hb_ps[:])
                nc.sync.dma_start(out=h_mine[b * P:(b + 1) * P, :],
                                  in_=h_loc[:, b * D:(b + 1) * D])
            tc.strict_bb_all_engine_barrier()
            nc.gpsimd.collective_compute(
                "AllGather", Alu.bypass,
                replica_groups=[list(range(NCORES))],
                ins=[h_mine.ap().opt()], outs=[h_full[0].ap().opt()])
            tc.strict_bb_all_engine_barrier()

            for l in range(L):
                mt_sb = cp.tile([D + 1, H * XW], f32, tag="mt")
                nc.sync.dma_start(out=mt_sb[:], in_=mt_in[l])
                wv2_sb = cp.tile([XW, H * D], f32, tag="wv2")
                nc.sync.dma_start(out=wv2_sb[:], in_=wv2_in[l])
                wsk_sb = cp.tile([D + 1, D], f32, tag="wsk")
                nc.sync.dma_start(out=wsk_sb[:], in_=wsk_in[l])

                for b in range(NB):
                    # ---- B1: per-block C matrix + skip into out PSUM
                    hT_ps = ps_t.tile([D, P], f32, tag="tr")
                    nc.tensor.transpose(out=hT_ps[:], in_=h_loc[:, b * D:(b + 1) * D],
                                        identity=ident[:])
                    hTa = xp.tile([D + 1, P], f32, tag="hta")
                    nc.vector.memset(hTa[:], 1.0)
                    nc.vector.tensor_copy(hTa[:D, :], hT_ps[:])
                    C_ps = ps_c.tile([XW, HD], f32, tag="C")
                    for h in range(H):
                        nc.tensor.matmul(C_ps[:, h * P:(h + 1) * P],
                                         lhsT=mt_sb[:, h * XW:(h + 1) * XW],
                                         rhs=hTa[:], start=True, stop=True,
                                         skip_group_check=(h > 0))
                    C_sb = vp.tile([XW, HD], f32, tag="C")
                    nc.vector.tensor_copy(C_sb[:], C_ps[:])
                    out_ps = ps_b.tile([P, D], f32, tag="out")
                    nc.tensor.matmul(out_ps[:], lhsT=hTa[:], rhs=wsk_sb[:],
                                     start=True, stop=False)

                    # ---- B2: edge tiles
                    agg_ps = ps_g.tile([XW, HD], f32, tag="agg")
                    for t in range(tpb):
                        g = (b * tpb + t) * SW
                        X = xp.tile([P, XW], f32, tag="X")
                        nc.gpsimd.indirect_dma_start(
                            out=X[:, :D], out_offset=None,
                            in_=h_full[l][:],
                            in_offset=bass.IndirectOffsetOnAxis(
                                ap=idxs[:, b * tpb + t:b * tpb + t + 1], axis=0))
                        nc.vector.tensor_copy(X[:, D:], eas[:, g:g + SW])
                        XT_ps = ps_t.tile([XW, P], f32, tag="tr")
                        nc.tensor.transpose(out=XT_ps[:], in_=X[:], identity=ident[:])
                        XT = xp.tile([XW, P], f32, tag="XT")
                        nc.vector.tensor_copy(XT[:], XT_ps[:])
                        al_ps = ps_a.tile([P, HD], f32, tag="al")
                        nc.tensor.matmul(al_ps[:], lhsT=XT[:], rhs=C_sb[:],
                                         start=True, stop=True)
                        S = vp.tile([P, P], f32, tag="S")
                        nc.gpsimd.tensor_scalar(out=S[:], in0=iota_f[:],
                                                scalar1=met[:, b * tpb + t:
                                                            b * tpb + t + 1],
                                                scalar2=None, op0=Alu.is_equal)
                        EX = vp.tile([P, HD], f32, tag="EX")
                        nc.scalar.activation(EX[:], al_ps[:], Act.Exp)
                        EXM = vp.tile([P, HD], f32, tag="EXM")
                        nc.vector.tensor_tensor(
                            out=EXM[:].rearrange("p (h d) -> p h d", h=H),
                            in0=EX[:].rearrange("p (h d) -> p h d", h=H),
                            in1=S[:, None, :].broadcast_to([P, H, P]),
                            op=Alu.mult)
                        nc.tensor.matmul(agg_ps[:], lhsT=X[:], rhs=EXM[:],
                                         start=(t == 0), stop=(t == tpb - 1))

                    # ---- B3: normalize, project, skip+relu
                    den = vp.tile([1, HD], f32, tag="den")
                    nc.vector.tensor_scalar_max(out=den[:], in0=agg_ps[D:D + 1, :],
                                                scalar1=1e-30)
                    rden = vp.tile([1, HD], f32, tag="rd")
                    nc.vector.reciprocal(rden[:], den[:])
                    rf_ps = ps_c.tile([XW, HD], f32, tag="C")
                    nc.tensor.matmul(rf_ps[:], lhsT=ones_row[:], rhs=rden[:],
                                     start=True, stop=True)
                    rfull = vp.tile([XW, HD], f32, tag="rf")
                    nc.vector.tensor_copy(rfull[:], rf_ps[:])
                    Zn = vp.tile([XW, HD], f32, tag="Zn")
                    nc.vector.tensor_tensor(out=Zn[:], in0=agg_ps[:], in1=rfull[:],
                                            op=Alu.mult)
                    for h in range(H):
                        nc.tensor.matmul(out_ps[:], lhsT=Zn[:, h * P:(h + 1) * P],
                                         rhs=wv2_sb[:, h * D:(h + 1) * D],
                                         start=False, stop=(h == H - 1))
                    nc.vector.tensor_scalar_max(
                        out=h_loc[:, b * D:(b + 1) * D], in0=out_ps[:], scalar1=0.0)
                    if l < L - 1:
                        nc.sync.dma_start(out=h_mine[b * P:(b + 1) * P, :],
                                          in_=h_loc[:, b * D:(b + 1) * D])
                if l < L - 1:
                    tc.strict_bb_all_engine_barrier()
                    nc.gpsimd.collective_compute(
                        "AllGather", Alu.bypass,
                        replica_groups=[list(range(NCORES))],
                        ins=[h_mine.ap().opt()], outs=[h_full[l + 1].ap().opt()])
                    tc.strict_bb_all_engine_barrier()

            # ---- pooling: one-hot on batch ids
            brel = cp.tile([P, NB], f32)
            nc.sync.dma_start(out=brel[:],
                              in_=brel_in[:].rearrange("(b p) o -> p (b o)", p=P))
            pool_ps = ps_a.tile([P, D], f32, tag="al")
            cnt_ps = ps_b.tile([P, 1], f32, tag="cnt")
            for b in range(NB):
                Sb = vp.tile([P, P], f32, tag="S")
                nc.vector.tensor_scalar(out=Sb[:], in0=iota_f[:],
                                        scalar1=brel[:, b:b + 1], scalar2=None,
                                        op0=Alu.is_equal)
                nc.tensor.matmul(pool_ps[:], lhsT=Sb[:],
                                 rhs=h_loc[:, b * D:(b + 1) * D],
                                 start=(b == 0), stop=(b == NB - 1))
                nc.tensor.matmul(cnt_ps[:], lhsT=Sb[:], rhs=ones_col[:],
                                 start=(b == 0), stop=(b == NB - 1),
                                 skip_group_check=True)
            pool_sb = vp.tile([P, D + 1], f32, tag="pool_sb")
            nc.vector.tensor_copy(pool_sb[:, :D], pool_ps[:])
            nc.vector.tensor_copy(pool_sb[:, D:], cnt_ps[:])
            nc.sync.dma_start(out=out_pool[:], in_=pool_sb[:])
    return nc


# --------------------------------------------------------------------- host --
def kernel(**inputs):
    _install_birpatch()
    from concourse.bass_utils import run_bass_kernel_spmd
    import ml_dtypes

    x = np.asarray(inputs["x"], np.float32)
    ei = np.asarray(inputs["edge_index"]).astype(np.int64)
    ea = np.asarray(inputs["edge_attr"], np.float32)
    batch = np.asarray(inputs["batch"]).astype(np.int64)
    Wq = np.asarray(inputs["Wq"], np.float32); bq = np.asarray(inputs["bq"], np.float32)
    Wk = np.asarray(inputs["Wk"], np.float32); bk = np.asarray(inputs["bk"], np.float32)
    Wv = np.asarray(inputs["Wv"], np.float32); bv = np.asarray(inputs["bv"], np.float32)
    We = np.asarray(inputs["We"], np.float32)
    Wskip = np.asarray(inputs["Wskip"], np.float32)
    bskip = np.asarray(inputs["bskip"], np.float32)
    W_atom = np.asarray(inputs["W_atom"], np.float32)
    b_atom = np.asarray(inputs["b_atom"], np.float32)
    W_edge = np.asarray(inputs["W_edge"], np.float32)
    b_edge = np.asarray(inputs["b_edge"], np.float32)
    W_out = np.asarray(inputs["W_out"], np.float32)
    b_out = np.asarray(inputs["b_out"], np.float32)

    src, dst = ei[0], ei[1]
    order = np.argsort(dst, kind="stable")
    src_s, dst_s = src[order], dst[order]
    ea_s = ea[order]

    # per-(core, block) edge ranges; uniform tile count tpb across all
    blk_of = dst_s // P                       # 0..156 (20 blocks x 8 cores)
    nblk = NCORES * NB
    counts = np.bincount(blk_of, minlength=nblk)
    starts = np.zeros(nblk + 1, np.int64)
    np.cumsum(counts, out=starts[1:])
    tpb = int(np.ceil(max(1, counts.max()) / P))
    NT = NB * tpb

    # folds: w2k rows = [Wk ; ones-row (bk + edge-bias) ; Wea@We], per layer
    Wea = np.concatenate([W_edge, b_edge[None, :]], 0)        # [51, 64]
    mt = np.zeros((L, D + 1, H * XW), np.float32)
    wv2 = np.zeros((L, H, XW, D), np.float32)
    wska = np.zeros((L, D + 1, D), np.float32)
    scale = 1.0 / np.sqrt(D)
    for l in range(L):
        ew = Wea @ We[l]                                      # [51, 256]
        w2k = np.zeros((XW, H * D), np.float32)
        w2k[:D] = Wk[l]
        w2k[D] = ew[DE] + bk[l]
        w2k[D + 1:] = ew[:DE]
        Wq_aug = np.concatenate([Wq[l], bq[l][None, :]], 0)   # [65, 256]
        for h in range(H):
            mt[l, :, h * XW:(h + 1) * XW] = (
                Wq_aug[:, h * D:(h + 1) * D] @ w2k[:, h * D:(h + 1) * D].T) * scale
            wv2[l, h, :D] = Wv[l][:, h * D:(h + 1) * D] / H
            wv2[l, h, D] = (ew[DE, h * D:(h + 1) * D]
                            + bv[l][h * D:(h + 1) * D]) / H
            wv2[l, h, D + 1:] = ew[:DE, h * D:(h + 1) * D] / H
        wska[l, :D] = Wskip[l]
        wska[l, D] = bskip[l]
    watom = np.concatenate([W_atom, b_atom[None, :]], 0)
    wv2 = np.ascontiguousarray(np.transpose(wv2, (0, 2, 1, 3)).reshape(L, XW, H * D))

    # vectorized slab build: rank within dst-block -> (tile, partition) slot
    rank = np.arange(E, dtype=np.int64) - starts[blk_of]
    tt, pp = rank // P, rank % P
    eslab = np.zeros((NCORES, P, NB, tpb, SW), np.float32)
    eslab[:, :, :, :, 0] = 1.0
    mslab = np.full((NCORES, P, NB, tpb), -1.0, np.float32)
    islab = np.zeros((NCORES, P, NB, tpb), np.int32)
    cc, bb = blk_of // NB, blk_of % NB
    mslab[cc, pp, bb, tt] = (dst_s - blk_of * P).astype(np.float32)
    eslab[cc, pp, bb, tt, 1:] = ea_s
    islab[cc, pp, bb, tt] = src_s
    eslab = eslab.reshape(NCORES, P, NT * SW).astype(ml_dtypes.bfloat16)
    mslab = mslab.reshape(NCORES, P, NT)
    islab = islab.reshape(NCORES, P, NT)

    in_maps, g0s = [], []
    for c in range(NCORES):
        n0 = c * NLOC
        real = min(NLOC, max(0, N - n0))
        xaugT = np.zeros((DA + 1, NLOC), np.float32)
        xaugT[DA] = 1.0
        xaugT[:DA, :real] = x[n0:n0 + real].T
        brel = np.full((NLOC, 1), -1.0, np.float32)
        g0 = int(batch[min(n0, N - 1)]) if n0 < N else 0
        if real > 0:
            brel[:real, 0] = batch[n0:n0 + real] - g0
        g0s.append(g0)
        in_maps.append({
            "xaugT": xaugT,
            "ea_slab": eslab[c],
            "met_slab": mslab[c],
            "idx_slab": islab[c],
            "batch_rel": brel,
            "w_atom_aug": watom, "mt": mt, "wv2": wv2, "wska": wska,
        })

    nc = _build_nc(tpb)
    res = run_bass_kernel_spmd(nc, in_maps, core_ids=list(range(NCORES)))

    sums = np.zeros((G + P, D), np.float64)
    cnts = np.zeros(G + P, np.float64)
    for c in range(NCORES):
        op = res.results[c]["out_pool"]
        sums[g0s[c]:g0s[c] + P] += op[:, :D]
        cnts[g0s[c]:g0s[c] + P] += op[:, D]
    pooled = sums[:G] / np.maximum(cnts[:G], 1.0)[:, None]
    out = pooled.astype(np.float32) @ W_out + b_out
    return out.squeeze()


# revision 27
# speedup vs baseline: 28.2980x; 1.1888x over previous
"""CrystalTransformer (TransformerConv x3 + segment-mean pool) on 8 trn2 cores.

Host: sort edges by dst, shard nodes into 8 contiguous 2560-node ranges
(128-aligned, zero-padded to 20480), pad per-dst-block edge lists to a uniform
tile count tpb so all 8 cores run one SPMD program.

Device per core/layer/block (128 dst nodes):
  B1: C = [M_h @ hT_aug]_h  (one [115, 512] matrix per block; M_h =
      w2k_h @ Wq_aug_h^T / sqrt(D) is a host-folded layer constant, so q is
      never materialized), skip = hT_aug^T @ Wskip_aug into the out PSUM.
  B2 per 128-edge tile: gather h[src] (indirect DMA) into X=[h_src|ea|1],
      XT = transpose(X), alphaT = XT^T @ C ([e, h*128+dst], all heads, one
      matmul), EXM = exp(alphaT) * S (S = per-edge one-hot of dst_rel,
      broadcast over heads), aggT += X^T @ EXM ([115, 512], one matmul).
  B3: den = aggT row 114 (ones-column trick), Zn = aggT * (1/den) via
      partition-broadcast, out += [Zn_h^T @ wv2_h]_h, h = relu(out).
AllGather h between layers; pooling via one-hot matmul on batch ids; final
tiny matmul on host. Edge slab ships as bf16 (halves upload), compute f32.
"""
import json
import numpy as np

P = 128
N, E, G = 20000, 320000, 256
DA, DE, D, H, L = 92, 50, 64, 4, 3
NCORES = 8
NLOC = 2560            # node slots per core (20 blocks of 128)
NB = NLOC // P         # 20 dst blocks per core
NPAD = NLOC * NCORES   # 20480
XW = D + DE + 1        # 115 = [h_src(64) | 1 | ea(50)]; ones at aligned row 64
SW = DE + 1            # 51 slab cols per tile: [1 | ea(50)]
HD = H * P             # 512 = heads * dst concatenated


# ---------------------------------------------------------------- BIR patch --
def _install_birpatch():
    """This container's walrus rejects >1 sem wait per instruction; hoist
    extras onto injected preceding Drains (same engine => same order)."""
    import concourse.bass2jax as b2j
    if getattr(b2j, "_birpatch_installed", False):
        return
    orig = b2j.compile_bir_kernel

    def patch(bir_bytes):
        d = json.loads(bir_bytes)
        for fn in d.get("functions", []):
            for blk in fn.get("blocks", []):
                out = []
                for ins in blk.get("instructions", []):
                    si = ins.get("sync_info") or {}
                    waits = si.get("on_wait") or []
                    if len(waits) > 1:
                        for k, w in enumerate(waits[:-1]):
                            out.append({
                                "debug": ins.get("debug", 0),
                                "engine": ins["engine"], "ins": [], "outs": [],
                                "name": f'{ins["name"]}-w{k}', "opcode": "Drain",
                                "sync_info": {"on_update": [], "on_wait": [w]},
                            })
                        si["on_wait"] = waits[-1:]
                    out.append(ins)
                blk["instructions"] = out
        return json.dumps(d).encode()

    def wrapper(bir_str, *a, **kw):
        try:
            bir_str = patch(bir_str)
        except Exception as e:  # pragma: no cover
            print("[birpatch] failed:", e)
        return orig(bir_str, *a, **kw)

    b2j.compile_bir_kernel = wrapper
    b2j._birpatch_installed = True


# ------------------------------------------------------------------- device --
def _build_nc(tpb):
    import concourse.bass as bass
    import concourse.mybir as mybir
    import concourse.tile as tile
    from concourse.masks import make_identity

    f32, i32, bf16 = mybir.dt.float32, mybir.dt.int32, mybir.dt.bfloat16
    Alu, Act = mybir.AluOpType, mybir.ActivationFunctionType
    NT = NB * tpb          # edge tiles per core

    nc = bass.Bass("TRN2", target_bir_lowering=False, debug=False,
                   num_devices=NCORES)
    di = lambda nm, sh, dt=f32: nc.dram_tensor(nm, sh, dt, kind="ExternalInput")
    xaug_in = di("xaugT", [DA + 1, NLOC])
    eas_in = di("ea_slab", [P, NT * SW], bf16)   # [ea | 1] per tile
    met_in = di("met_slab", [P, NT])             # dst_rel per tile (f32)
    idx_in = di("idx_slab", [P, NT], i32)        # src_global per tile
    brel_in = di("batch_rel", [NLOC, 1])
    watom_in = di("w_atom_aug", [DA + 1, D])
    mt_in = di("mt", [L, D + 1, H * XW], bf16)   # [Wq_aug_h @ w2k_h^T / 8]_h
    wv2_in = di("wv2", [L, XW, H * D], bf16)
    wsk_in = di("wska", [L, D + 1, D], bf16)
    out_pool = nc.dram_tensor("out_pool", [P, D + 1], f32, kind="ExternalOutput")

    h_mine = nc.dram_tensor("h_mine", [NLOC, D], bf16)
    h_full = [nc.dram_tensor(f"h_full_{l}", [NPAD, D], bf16, addr_space="Shared")
              for l in range(L)]

    with tile.TileContext(nc, num_cores=NCORES) as tc:
        import contextlib
        with contextlib.ExitStack() as st:
            cp = st.enter_context(tc.tile_pool(name="const", bufs=1))
            xp = st.enter_context(tc.tile_pool(name="xt", bufs=3))
            vp = st.enter_context(tc.tile_pool(name="dve", bufs=3))
            ps_t = st.enter_context(tc.tile_pool(name="ps_t", bufs=1, space="PSUM"))
            ps_a = st.enter_context(tc.tile_pool(name="ps_a", bufs=2, space="PSUM"))
            ps_g = st.enter_context(tc.tile_pool(name="ps_g", bufs=2, space="PSUM"))
            ps_c = st.enter_context(tc.tile_pool(name="ps_c", bufs=1, space="PSUM"))
            ps_b = st.enter_context(tc.tile_pool(name="ps_b", bufs=1, space="PSUM"))

            ident = cp.tile([P, P], f32)
            make_identity(nc, ident[:])
            ident_bf = cp.tile([P, P], bf16)
            nc.vector.tensor_copy(ident_bf[:], ident[:])
            iota_i = cp.tile([P, P], i32)
            nc.gpsimd.iota(iota_i[:], pattern=[[1, P]], base=0, channel_multiplier=0)
            iota_f = cp.tile([P, P], f32)
            nc.vector.tensor_copy(iota_f[:], iota_i[:])
            ones_col = cp.tile([P, 1], bf16)
            nc.vector.memset(ones_col[:], 1.0)
            ones_row = cp.tile([1, XW], f32)
            nc.vector.memset(ones_row[:], 1.0)
            h_loc = cp.tile([P, NB * D], bf16)
            watom_sb = cp.tile([DA + 1, D], f32)
            nc.sync.dma_start(out=watom_sb[:], in_=watom_in[:])
            met = cp.tile([P, NT], f32)
            nc.sync.dma_start(out=met[:], in_=met_in[:])
            idxs = cp.tile([P, NT], i32)
            nc.sync.dma_start(out=idxs[:], in_=idx_in[:])

            # ---- embed: h0 = x@W_atom + b (no relu, as in reference)
            for b in range(NB):
                xT = xp.tile([DA + 1, P], f32, tag="hta")
                nc.sync.dma_start(out=xT[:], in_=xaug_in[:, b * P:(b + 1) * P])
                hb_ps = ps_b.tile([P, D], f32, tag="out")
                nc.tensor.matmul(hb_ps[:], lhsT=xT[:], rhs=watom_sb[:],
                                 start=True, stop=True)
                nc.vector.tensor_copy(h_loc[:, b * D:(b + 1) * D], hb_ps[:])
                nc.sync.dma_start(out=h_mine[b * P:(b + 1) * P, :],
                                  in_=h_loc[:, b * D:(b + 1) * D])
            tc.strict_bb_all_engine_barrier()
            nc.gpsimd.collective_compute(
                "AllGather", Alu.bypass,
                replica_groups=[list(range(NCORES))],
                ins=[h_mine.ap().opt()], outs=[h_full[0].ap().opt()])
            tc.strict_bb_all_engine_barrier()

            for l in range(L):
                mt_sb = cp.tile([D + 1, H * XW], bf16, tag="mt")
                nc.sync.dma_start(out=mt_sb[:], in_=mt_in[l])
                wv2_sb = cp.tile([XW, H * D], bf16, tag="wv2")
                nc.sync.dma_start(out=wv2_sb[:], in_=wv2_in[l])
                wsk_sb = cp.tile([D + 1, D], bf16, tag="wsk")
                nc.sync.dma_start(out=wsk_sb[:], in_=wsk_in[l])

                for b in range(NB):
                    # ---- B1: per-block C matrix + skip into out PSUM
                    hT_ps = ps_t.tile([D, P], bf16, tag="tr")
                    nc.tensor.transpose(out=hT_ps[:], in_=h_loc[:, b * D:(b + 1) * D],
                                        identity=ident_bf[:])
                    hTa = xp.tile([D + 1, P], bf16, tag="hta")
                    nc.vector.memset(hTa[:], 1.0)
                    nc.vector.tensor_copy(hTa[:D, :], hT_ps[:])
                    C_ps = ps_c.tile([XW, HD], f32, tag="C")
                    for h in range(H):
                        nc.tensor.matmul(C_ps[:, h * P:(h + 1) * P],
                                         lhsT=mt_sb[:, h * XW:(h + 1) * XW],
                                         rhs=hTa[:], start=True, stop=True,
                                         skip_group_check=(h > 0))
                    C_sb = vp.tile([XW, HD], bf16, tag="C")
                    nc.vector.tensor_copy(C_sb[:], C_ps[:])
                    out_ps = ps_b.tile([P, D], f32, tag="out")
                    nc.tensor.matmul(out_ps[:], lhsT=hTa[:], rhs=wsk_sb[:],
                                     start=True, stop=False)

                    # ---- B2: edge tiles; X arena = [h_src | 1 | ea] per tile
                    Xa = xp.tile([P, tpb * XW], bf16, tag="X")
                    nc.sync.dma_start(
                        out=Xa[:].rearrange("p (t c) -> p t c", c=XW)[:, :, D:],
                        in_=eas_in[:, b * tpb * SW:(b + 1) * tpb * SW]
                        .rearrange("p (t c) -> p t c", c=SW))
                    agg_ps = ps_g.tile([XW, HD], f32, tag="agg")
                    for t in range(tpb):
                        X = Xa[:, t * XW:(t + 1) * XW]
                        nc.gpsimd.indirect_dma_start(
                            out=Xa[:, t * XW:t * XW + D], out_offset=None,
                            in_=h_full[l][:],
                            in_offset=bass.IndirectOffsetOnAxis(
                                ap=idxs[:, b * tpb + t:b * tpb + t + 1], axis=0))
                        XT_ps = ps_t.tile([XW, P], bf16, tag="tr")
                        nc.tensor.transpose(out=XT_ps[:], in_=X, identity=ident_bf[:])
                        XT = xp.tile([XW, P], bf16, tag="XT")
                        nc.vector.tensor_copy(XT[:], XT_ps[:])
                        al_ps = ps_a.tile([P, HD], f32, tag="al")
                        nc.tensor.matmul(al_ps[:], lhsT=XT[:], rhs=C_sb[:],
                                         start=True, stop=True)
                        S = vp.tile([P, P], bf16, tag="S")
                        nc.gpsimd.tensor_scalar(out=S[:], in0=iota_f[:],
                                                scalar1=met[:, b * tpb + t:
                                                            b * tpb + t + 1],
                                                scalar2=None, op0=Alu.is_equal)
                        EX = vp.tile([P, HD], bf16, tag="EX")
                        nc.scalar.activation(EX[:], al_ps[:], Act.Exp)
                        EXM = vp.tile([P, HD], bf16, tag="EXM")
                        nc.vector.tensor_tensor(
                            out=EXM[:].rearrange("p (h d) -> p h d", h=H),
                            in0=EX[:].rearrange("p (h d) -> p h d", h=H),
                            in1=S[:, None, :].broadcast_to([P, H, P]),
                            op=Alu.mult)
                        nc.tensor.matmul(agg_ps[:], lhsT=X, rhs=EXM[:],
                                         start=(t == 0), stop=(t == tpb - 1))

                    # ---- B3: normalize, project, skip+relu
                    den = vp.tile([1, HD], f32, tag="den")
                    nc.vector.tensor_scalar_max(out=den[:], in0=agg_ps[D:D + 1, :],
                                                scalar1=1e-30)
                    rden = vp.tile([1, HD], f32, tag="rd")
                    nc.vector.reciprocal(rden[:], den[:])
                    rf_ps = ps_c.tile([XW, HD], f32, tag="C")
                    nc.tensor.matmul(rf_ps[:], lhsT=ones_row[:], rhs=rden[:],
                                     start=True, stop=True)
                    rfull = vp.tile([XW, HD], f32, tag="rf")
                    nc.vector.tensor_copy(rfull[:], rf_ps[:])
                    Zn = vp.tile([XW, HD], bf16, tag="Zn")
                    nc.vector.tensor_tensor(out=Zn[:], in0=agg_ps[:], in1=rfull[:],
                                            op=Alu.mult)
                    for h in range(H):
                        nc.tensor.matmul(out_ps[:], lhsT=Zn[:, h * P:(h + 1) * P],
                                         rhs=wv2_sb[:, h * D:(h + 1) * D],
                                         start=False, stop=(h == H - 1))
                    nc.vector.tensor_scalar_max(
                        out=h_loc[:, b * D:(b + 1) * D], in0=out_ps[:], scalar1=0.0)
                    if l < L - 1:
                        nc.sync.dma_start(out=h_mine[b * P:(b + 1) * P, :],
                                          in_=h_loc[:, b * D:(b + 1) * D])
                if l < L - 1:
                    tc.strict_bb_all_engine_barrier()
                    nc.gpsimd.collective_compute(
                        "AllGather", Alu.bypass,
                        replica_groups=[list(range(NCORES))],
                        ins=[h_mine.ap().opt()], outs=[h_full[l + 1].ap().opt()])
                    tc.strict_bb_all_engine_barrier()

            # ---- pooling: one-hot on batch ids
            brel = cp.tile([P, NB], f32)
            nc.sync.dma_start(out=brel[:],
                              in_=brel_in[:].rearrange("(b p) o -> p (b o)", p=P))
            pool_ps = ps_a.tile([P, D], f32, tag="al")
            cnt_ps = ps_b.tile([P, 1], f32, tag="cnt")
            for b in range(NB):
                Sb = vp.tile([P, P], bf16, tag="S")
                nc.vector.tensor_scalar(out=Sb[:], in0=iota_f[:],
                                        scalar1=brel[:, b:b + 1], scalar2=None,
                                        op0=Alu.is_equal)
                nc.tensor.matmul(pool_ps[:], lhsT=Sb[:],
                                 rhs=h_loc[:, b * D:(b + 1) * D],
                                 start=(b == 0), stop=(b == NB - 1))
                nc.tensor.matmul(cnt_ps[:], lhsT=Sb[:], rhs=ones_col[:],
                                 start=(b == 0), stop=(b == NB - 1),
                                 skip_group_check=True)
            pool_sb = vp.tile([P, D + 1], f32, tag="pool_sb")
            nc.vector.tensor_copy(pool_sb[:, :D], pool_ps[:])
            nc.vector.tensor_copy(pool_sb[:, D:], cnt_ps[:])
            nc.sync.dma_start(out=out_pool[:], in_=pool_sb[:])
    return nc


# --------------------------------------------------------------------- host --
def kernel(**inputs):
    _install_birpatch()
    from concourse.bass_utils import run_bass_kernel_spmd
    import ml_dtypes

    x = np.asarray(inputs["x"], np.float32)
    ei = np.asarray(inputs["edge_index"]).astype(np.int64)
    ea = np.asarray(inputs["edge_attr"], np.float32)
    batch = np.asarray(inputs["batch"]).astype(np.int64)
    Wq = np.asarray(inputs["Wq"], np.float32); bq = np.asarray(inputs["bq"], np.float32)
    Wk = np.asarray(inputs["Wk"], np.float32); bk = np.asarray(inputs["bk"], np.float32)
    Wv = np.asarray(inputs["Wv"], np.float32); bv = np.asarray(inputs["bv"], np.float32)
    We = np.asarray(inputs["We"], np.float32)
    Wskip = np.asarray(inputs["Wskip"], np.float32)
    bskip = np.asarray(inputs["bskip"], np.float32)
    W_atom = np.asarray(inputs["W_atom"], np.float32)
    b_atom = np.asarray(inputs["b_atom"], np.float32)
    W_edge = np.asarray(inputs["W_edge"], np.float32)
    b_edge = np.asarray(inputs["b_edge"], np.float32)
    W_out = np.asarray(inputs["W_out"], np.float32)
    b_out = np.asarray(inputs["b_out"], np.float32)

    src, dst = ei[0], ei[1]
    order = np.argsort(dst, kind="stable")
    src_s, dst_s = src[order], dst[order]
    ea_s = ea[order]

    # per-(core, block) edge ranges; uniform tile count tpb across all
    blk_of = dst_s // P                       # 0..156 (20 blocks x 8 cores)
    nblk = NCORES * NB
    counts = np.bincount(blk_of, minlength=nblk)
    starts = np.zeros(nblk + 1, np.int64)
    np.cumsum(counts, out=starts[1:])
    tpb = int(np.ceil(max(1, counts.max()) / P))
    NT = NB * tpb

    # folds: w2k rows = [Wk ; ones-row (bk + edge-bias) ; Wea@We], per layer
    Wea = np.concatenate([W_edge, b_edge[None, :]], 0)        # [51, 64]
    mt = np.zeros((L, D + 1, H * XW), np.float32)
    wv2 = np.zeros((L, H, XW, D), np.float32)
    wska = np.zeros((L, D + 1, D), np.float32)
    scale = 1.0 / np.sqrt(D)
    for l in range(L):
        ew = Wea @ We[l]                                      # [51, 256]
        w2k = np.zeros((XW, H * D), np.float32)
        w2k[:D] = Wk[l]
        w2k[D] = ew[DE] + bk[l]
        w2k[D + 1:] = ew[:DE]
        Wq_aug = np.concatenate([Wq[l], bq[l][None, :]], 0)   # [65, 256]
        for h in range(H):
            mt[l, :, h * XW:(h + 1) * XW] = (
                Wq_aug[:, h * D:(h + 1) * D] @ w2k[:, h * D:(h + 1) * D].T) * scale
            wv2[l, h, :D] = Wv[l][:, h * D:(h + 1) * D] / H
            wv2[l, h, D] = (ew[DE, h * D:(h + 1) * D]
                            + bv[l][h * D:(h + 1) * D]) / H
            wv2[l, h, D + 1:] = ew[:DE, h * D:(h + 1) * D] / H
        wska[l, :D] = Wskip[l]
        wska[l, D] = bskip[l]
    watom = np.concatenate([W_atom, b_atom[None, :]], 0)
    wv2 = np.ascontiguousarray(np.transpose(wv2, (0, 2, 1, 3)).reshape(L, XW, H * D))

    # vectorized slab build: rank within dst-block -> (tile, partition) slot
    rank = np.arange(E, dtype=np.int64) - starts[blk_of]
    tt, pp = rank // P, rank % P
    eslab = np.zeros((NCORES, P, NB, tpb, SW), np.float32)
    eslab[:, :, :, :, 0] = 1.0
    mslab = np.full((NCORES, P, NB, tpb), -1.0, np.float32)
    islab = np.zeros((NCORES, P, NB, tpb), np.int32)
    cc, bb = blk_of // NB, blk_of % NB
    mslab[cc, pp, bb, tt] = (dst_s - blk_of * P).astype(np.float32)
    eslab[cc, pp, bb, tt, 1:] = ea_s
    islab[cc, pp, bb, tt] = src_s
    eslab = eslab.reshape(NCORES, P, NT * SW).astype(ml_dtypes.bfloat16)
    mslab = mslab.reshape(NCORES, P, NT)
    islab = islab.reshape(NCORES, P, NT)

    in_maps, g0s = [], []
    for c in range(NCORES):
        n0 = c * NLOC
        real = min(NLOC, max(0, N - n0))
        xaugT = np.zeros((DA + 1, NLOC), np.float32)
        xaugT[DA] = 1.0
        xaugT[:DA, :real] = x[n0:n0 + real].T
        brel = np.full((NLOC, 1), -1.0, np.float32)
        g0 = int(batch[min(n0, N - 1)]) if n0 < N else 0
        if real > 0:
            brel[:real, 0] = batch[n0:n0 + real] - g0
        g0s.append(g0)
        in_maps.append({
            "xaugT": xaugT,
            "ea_slab": eslab[c],
            "met_slab": mslab[c],
            "idx_slab": islab[c],
            "batch_rel": brel,
            "w_atom_aug": watom,
            "mt": mt.astype(ml_dtypes.bfloat16),
            "wv2": wv2.astype(ml_dtypes.bfloat16),
            "wska": wska.astype(ml_dtypes.bfloat16),
        })

    nc = _build_nc(tpb)
    res = run_bass_kernel_spmd(nc, in_maps, core_ids=list(range(NCORES)))

    sums = np.zeros((G + P, D), np.float64)
    cnts = np.zeros(G + P, np.float64)
    for c in range(NCORES):
        op = res.results[c]["out_pool"]
        sums[g0s[c]:g0s[c] + P] += op[:, :D]
        cnts[g0s[c]:g0s[c] + P] += op[:, D]
    pooled = sums[:G] / np.maximum(cnts[:G], 1.0)[:, None]
    out = pooled.astype(np.float32) @ W_out + b_out
    return out.squeeze()


# revision 30
# speedup vs baseline: 35.0350x; 1.2381x over previous
"""CrystalTransformer (TransformerConv x3 + segment-mean pool) on 8 trn2 cores.

Host: sort edges by dst, shard nodes into 8 contiguous 2560-node ranges
(128-aligned, zero-padded to 20480), pad per-dst-block edge lists to a uniform
tile count tpb so all 8 cores run one SPMD program.

Device per core/layer/block (128 dst nodes):
  B1: C = [M_h @ hT_aug]_h  (one [115, 512] matrix per block; M_h =
      w2k_h @ Wq_aug_h^T / sqrt(D) is a host-folded layer constant, so q is
      never materialized), skip = hT_aug^T @ Wskip_aug into the out PSUM.
  B2 per 128-edge tile: gather h[src] (indirect DMA) into X=[h_src|ea|1],
      XT = transpose(X), alphaT = XT^T @ C ([e, h*128+dst], all heads, one
      matmul), EXM = exp(alphaT) * S (S = per-edge one-hot of dst_rel,
      broadcast over heads), aggT += X^T @ EXM ([115, 512], one matmul).
  B3: den = aggT row 114 (ones-column trick), Zn = aggT * (1/den) via
      partition-broadcast, out += [Zn_h^T @ wv2_h]_h, h = relu(out).
AllGather h between layers; pooling via one-hot matmul on batch ids; final
tiny matmul on host. bf16 matmul inputs / h storage / edge slab (halves both
the upload and the PE-array streaming time); PSUM accumulation stays f32.
"""
import json
import numpy as np

P = 128
N, E, G = 20000, 320000, 256
DA, DE, D, H, L = 92, 50, 64, 4, 3
NCORES = 8
NLOC = 2560            # node slots per core (20 blocks of 128)
NB = NLOC // P         # 20 dst blocks per core
NPAD = NLOC * NCORES   # 20480
XW = D + DE + 1        # 115 = [h_src(64) | 1 | ea(50)]; ones at aligned row 64
SW = DE + 1            # 51 slab cols per tile: [1 | ea(50)]
HD = H * P             # 512 = heads * dst concatenated


# ---------------------------------------------------------------- BIR patch --
def _install_birpatch():
    """This container's walrus rejects >1 sem wait per instruction; hoist
    extras onto injected preceding Drains (same engine => same order)."""
    import concourse.bass2jax as b2j
    if getattr(b2j, "_birpatch_installed", False):
        return
    orig = b2j.compile_bir_kernel

    def patch(bir_bytes):
        d = json.loads(bir_bytes)
        for fn in d.get("functions", []):
            for blk in fn.get("blocks", []):
                out = []
                for ins in blk.get("instructions", []):
                    si = ins.get("sync_info") or {}
                    waits = si.get("on_wait") or []
                    if len(waits) > 1:
                        for k, w in enumerate(waits[:-1]):
                            out.append({
                                "debug": ins.get("debug", 0),
                                "engine": ins["engine"], "ins": [], "outs": [],
                                "name": f'{ins["name"]}-w{k}', "opcode": "Drain",
                                "sync_info": {"on_update": [], "on_wait": [w]},
                            })
                        si["on_wait"] = waits[-1:]
                    out.append(ins)
                blk["instructions"] = out
        return json.dumps(d).encode()

    def wrapper(bir_str, *a, **kw):
        try:
            bir_str = patch(bir_str)
        except Exception as e:  # pragma: no cover
            print("[birpatch] failed:", e)
        return orig(bir_str, *a, **kw)

    b2j.compile_bir_kernel = wrapper
    b2j._birpatch_installed = True


# ------------------------------------------------------------------- device --
def _build_nc(tpb):
    import concourse.bass as bass
    import concourse.mybir as mybir
    import concourse.tile as tile
    from concourse.masks import make_identity

    f32, i32, bf16 = mybir.dt.float32, mybir.dt.int32, mybir.dt.bfloat16
    Alu, Act = mybir.AluOpType, mybir.ActivationFunctionType
    NT = NB * tpb          # edge tiles per core

    nc = bass.Bass("TRN2", target_bir_lowering=False, debug=False,
                   num_devices=NCORES)
    di = lambda nm, sh, dt=f32: nc.dram_tensor(nm, sh, dt, kind="ExternalInput")
    xaug_in = di("xaugT", [DA + 1, NLOC])
    eas_in = di("ea_slab", [P, NT * SW], bf16)   # [ea | 1] per tile
    met_in = di("met_slab", [P, NT])             # dst_rel per tile (f32)
    idx_in = di("idx_slab", [P, NT], i32)        # src_global per tile
    brel_in = di("batch_rel", [NLOC, 1])
    watom_in = di("w_atom_aug", [DA + 1, D])
    mt_in = di("mt", [L, D + 1, H * XW], bf16)   # [Wq_aug_h @ w2k_h^T / 8]_h
    wv2_in = di("wv2", [L, XW, H * D], bf16)
    wsk_in = di("wska", [L, D + 1, D], bf16)
    out_pool = nc.dram_tensor("out_pool", [P, D + 1], f32, kind="ExternalOutput")

    h_mine = nc.dram_tensor("h_mine", [NLOC, D], bf16)
    h_full = [nc.dram_tensor(f"h_full_{l}", [NPAD, D], bf16, addr_space="Shared")
              for l in range(L)]

    with tile.TileContext(nc, num_cores=NCORES) as tc:
        import contextlib
        with contextlib.ExitStack() as st:
            cp = st.enter_context(tc.tile_pool(name="const", bufs=1))
            xp = st.enter_context(tc.tile_pool(name="xt", bufs=3))
            vp = st.enter_context(tc.tile_pool(name="dve", bufs=3))
            ps_t = st.enter_context(tc.tile_pool(name="ps_t", bufs=1, space="PSUM"))
            ps_a = st.enter_context(tc.tile_pool(name="ps_a", bufs=2, space="PSUM"))
            ps_g = st.enter_context(tc.tile_pool(name="ps_g", bufs=2, space="PSUM"))
            ps_c = st.enter_context(tc.tile_pool(name="ps_c", bufs=1, space="PSUM"))
            ps_b = st.enter_context(tc.tile_pool(name="ps_b", bufs=1, space="PSUM"))

            ident = cp.tile([P, P], f32)
            make_identity(nc, ident[:])
            ident_bf = cp.tile([P, P], bf16)
            nc.vector.tensor_copy(ident_bf[:], ident[:])
            iota_i = cp.tile([P, P], i32)
            nc.gpsimd.iota(iota_i[:], pattern=[[1, P]], base=0, channel_multiplier=0)
            iota_f = cp.tile([P, P], f32)
            nc.vector.tensor_copy(iota_f[:], iota_i[:])
            ones_col = cp.tile([P, 1], bf16)
            nc.vector.memset(ones_col[:], 1.0)
            ones_row = cp.tile([1, XW], f32)
            nc.vector.memset(ones_row[:], 1.0)
            h_loc = cp.tile([P, NB * D], bf16)
            watom_sb = cp.tile([DA + 1, D], f32)
            nc.sync.dma_start(out=watom_sb[:], in_=watom_in[:])
            met = cp.tile([P, NT], f32)
            nc.sync.dma_start(out=met[:], in_=met_in[:])
            idxs = cp.tile([P, NT], i32)
            nc.sync.dma_start(out=idxs[:], in_=idx_in[:])

            # ---- embed: h0 = x@W_atom + b (no relu, as in reference)
            for b in range(NB):
                xT = xp.tile([DA + 1, P], f32, tag="hta")
                nc.sync.dma_start(out=xT[:], in_=xaug_in[:, b * P:(b + 1) * P])
                hb_ps = ps_b.tile([P, D], f32, tag="out")
                nc.tensor.matmul(hb_ps[:], lhsT=xT[:], rhs=watom_sb[:],
                                 start=True, stop=True)
                nc.vector.tensor_copy(h_loc[:, b * D:(b + 1) * D], hb_ps[:])
                nc.sync.dma_start(out=h_mine[b * P:(b + 1) * P, :],
                                  in_=h_loc[:, b * D:(b + 1) * D])
            tc.strict_bb_all_engine_barrier()
            nc.gpsimd.collective_compute(
                "AllGather", Alu.bypass,
                replica_groups=[list(range(NCORES))],
                ins=[h_mine.ap().opt()], outs=[h_full[0].ap().opt()])
            tc.strict_bb_all_engine_barrier()

            for l in range(L):
                mt_sb = cp.tile([D + 1, H * XW], bf16, tag="mt")
                nc.sync.dma_start(out=mt_sb[:], in_=mt_in[l])
                wv2_sb = cp.tile([XW, H * D], bf16, tag="wv2")
                nc.sync.dma_start(out=wv2_sb[:], in_=wv2_in[l])
                wsk_sb = cp.tile([D + 1, D], bf16, tag="wsk")
                nc.sync.dma_start(out=wsk_sb[:], in_=wsk_in[l])

                for b in range(NB):
                    # ---- B1: per-block C matrix + skip into out PSUM
                    hT_ps = ps_t.tile([D, P], bf16, tag="tr")
                    nc.tensor.transpose(out=hT_ps[:], in_=h_loc[:, b * D:(b + 1) * D],
                                        identity=ident_bf[:])
                    hTa = xp.tile([D + 1, P], bf16, tag="hta")
                    nc.vector.memset(hTa[:], 1.0)
                    nc.vector.tensor_copy(hTa[:D, :], hT_ps[:])
                    C_ps = ps_c.tile([XW, HD], f32, tag="C")
                    for h in range(H):
                        nc.tensor.matmul(C_ps[:, h * P:(h + 1) * P],
                                         lhsT=mt_sb[:, h * XW:(h + 1) * XW],
                                         rhs=hTa[:], start=True, stop=True,
                                         skip_group_check=(h > 0))
                    C_sb = vp.tile([XW, HD], bf16, tag="C")
                    nc.vector.tensor_copy(C_sb[:], C_ps[:])
                    out_ps = ps_b.tile([P, D], f32, tag="out")
                    nc.tensor.matmul(out_ps[:], lhsT=hTa[:], rhs=wsk_sb[:],
                                     start=True, stop=False)

                    # ---- B2: edge tiles; X arena = [h_src | 1 | ea] per tile
                    Xa = xp.tile([P, tpb * XW], bf16, tag="X")
                    nc.sync.dma_start(
                        out=Xa[:].rearrange("p (t c) -> p t c", c=XW)[:, :, D:],
                        in_=eas_in[:, b * tpb * SW:(b + 1) * tpb * SW]
                        .rearrange("p (t c) -> p t c", c=SW))
                    agg_ps = ps_g.tile([XW, HD], f32, tag="agg")
                    for t in range(tpb):
                        X = Xa[:, t * XW:(t + 1) * XW]
                        nc.gpsimd.indirect_dma_start(
                            out=Xa[:, t * XW:t * XW + D], out_offset=None,
                            in_=h_full[l][:],
                            in_offset=bass.IndirectOffsetOnAxis(
                                ap=idxs[:, b * tpb + t:b * tpb + t + 1], axis=0))
                        XT_ps = ps_t.tile([XW, P], bf16, tag="tr")
                        nc.tensor.transpose(out=XT_ps[:], in_=X, identity=ident_bf[:])
                        XT = xp.tile([XW, P], bf16, tag="XT")
                        nc.vector.tensor_copy(XT[:], XT_ps[:])
                        al_ps = ps_a.tile([P, HD], f32, tag="al")
                        nc.tensor.matmul(al_ps[:], lhsT=XT[:], rhs=C_sb[:],
                                         start=True, stop=True)
                        S = vp.tile([P, P], bf16, tag="S")
                        nc.gpsimd.tensor_scalar(out=S[:], in0=iota_f[:],
                                                scalar1=met[:, b * tpb + t:
                                                            b * tpb + t + 1],
                                                scalar2=None, op0=Alu.is_equal)
                        EX = vp.tile([P, HD], bf16, tag="EX")
                        nc.scalar.activation(EX[:], al_ps[:], Act.Exp)
                        EXM = vp.tile([P, HD], bf16, tag="EXM")
                        nc.vector.tensor_tensor(
                            out=EXM[:].rearrange("p (h d) -> p h d", h=H),
                            in0=EX[:].rearrange("p (h d) -> p h d", h=H),
                            in1=S[:, None, :].broadcast_to([P, H, P]),
                            op=Alu.mult)
                        nc.tensor.matmul(agg_ps[:], lhsT=X, rhs=EXM[:],
                                         start=(t == 0), stop=(t == tpb - 1))

                    # ---- B3: normalize, project, skip+relu
                    den = vp.tile([1, HD], f32, tag="den")
                    nc.vector.tensor_scalar_max(out=den[:], in0=agg_ps[D:D + 1, :],
                                                scalar1=1e-30)
                    rden = vp.tile([1, HD], f32, tag="rd")
                    nc.vector.reciprocal(rden[:], den[:])
                    rf_ps = ps_c.tile([XW, HD], f32, tag="C")
                    nc.tensor.matmul(rf_ps[:], lhsT=ones_row[:], rhs=rden[:],
                                     start=True, stop=True)
                    rfull = vp.tile([XW, HD], f32, tag="rf")
                    nc.vector.tensor_copy(rfull[:], rf_ps[:])
                    Zn = vp.tile([XW, HD], bf16, tag="Zn")
                    nc.vector.tensor_tensor(out=Zn[:], in0=agg_ps[:], in1=rfull[:],
                                            op=Alu.mult)
                    for h in range(H):
                        nc.tensor.matmul(out_ps[:], lhsT=Zn[:, h * P:(h + 1) * P],
                                         rhs=wv2_sb[:, h * D:(h + 1) * D],
                                         start=False, stop=(h == H - 1))
                    nc.vector.tensor_scalar_max(
                        out=h_loc[:, b * D:(b + 1) * D], in0=out_ps[:], scalar1=0.0)
                    if l < L - 1:
                        nc.sync.dma_start(out=h_mine[b * P:(b + 1) * P, :],
                                          in_=h_loc[:, b * D:(b + 1) * D])
                if l < L - 1:
                    tc.strict_bb_all_engine_barrier()
                    nc.gpsimd.collective_compute(
                        "AllGather", Alu.bypass,
                        replica_groups=[list(range(NCORES))],
                        ins=[h_mine.ap().opt()], outs=[h_full[l + 1].ap().opt()])
                    tc.strict_bb_all_engine_barrier()

            # ---- pooling: one-hot on batch ids
            brel = cp.tile([P, NB], f32)
            nc.sync.dma_start(out=brel[:],
                              in_=brel_in[:].rearrange("(b p) o -> p (b o)", p=P))
            pool_ps = ps_a.tile([P, D], f32, tag="al")
            cnt_ps = ps_b.tile([P, 1], f32, tag="cnt")
            for b in range(NB):
                Sb = vp.tile([P, P], bf16, tag="S")
                nc.vector.tensor_scalar(out=Sb[:], in0=iota_f[:],
                                        scalar1=brel[:, b:b + 1], scalar2=None,
                                        op0=Alu.is_equal)
                nc.tensor.matmul(pool_ps[:], lhsT=Sb[:],
                                 rhs=h_loc[:, b * D:(b + 1) * D],
                                 start=(b == 0), stop=(b == NB - 1))
                nc.tensor.matmul(cnt_ps[:], lhsT=Sb[:], rhs=ones_col[:],
                                 start=(b == 0), stop=(b == NB - 1),
                                 skip_group_check=True)
            pool_sb = vp.tile([P, D + 1], f32, tag="pool_sb")
            nc.vector.tensor_copy(pool_sb[:, :D], pool_ps[:])
            nc.vector.tensor_copy(pool_sb[:, D:], cnt_ps[:])
            nc.sync.dma_start(out=out_pool[:], in_=pool_sb[:])
    return nc


# --------------------------------------------------------------------- host --
_NC_CACHE = {}


def _get_nc(tpb):
    if tpb not in _NC_CACHE:
        _NC_CACHE[tpb] = _build_nc(tpb)
    return _NC_CACHE[tpb]


def kernel(**inputs):
    _install_birpatch()
    import threading
    # tpb is data-dependent but 17 for the reference distribution; build the
    # Bass program speculatively while numpy preps the shards (Bass() pays a
    # one-time ~1s ISA/cffi setup that doesn't need the data).
    th = threading.Thread(target=lambda: _get_nc(17), daemon=True)
    th.start()
    from concourse.bass_utils import run_bass_kernel_spmd
    import ml_dtypes

    x = np.asarray(inputs["x"], np.float32)
    ei = np.asarray(inputs["edge_index"]).astype(np.int64)
    ea = np.asarray(inputs["edge_attr"], np.float32)
    batch = np.asarray(inputs["batch"]).astype(np.int64)
    Wq = np.asarray(inputs["Wq"], np.float32); bq = np.asarray(inputs["bq"], np.float32)
    Wk = np.asarray(inputs["Wk"], np.float32); bk = np.asarray(inputs["bk"], np.float32)
    Wv = np.asarray(inputs["Wv"], np.float32); bv = np.asarray(inputs["bv"], np.float32)
    We = np.asarray(inputs["We"], np.float32)
    Wskip = np.asarray(inputs["Wskip"], np.float32)
    bskip = np.asarray(inputs["bskip"], np.float32)
    W_atom = np.asarray(inputs["W_atom"], np.float32)
    b_atom = np.asarray(inputs["b_atom"], np.float32)
    W_edge = np.asarray(inputs["W_edge"], np.float32)
    b_edge = np.asarray(inputs["b_edge"], np.float32)
    W_out = np.asarray(inputs["W_out"], np.float32)
    b_out = np.asarray(inputs["b_out"], np.float32)

    src, dst = ei[0], ei[1]
    order = np.argsort(dst, kind="stable")
    src_s, dst_s = src[order], dst[order]
    ea_s = ea[order]

    # per-(core, block) edge ranges; uniform tile count tpb across all
    blk_of = dst_s // P                       # 0..156 (20 blocks x 8 cores)
    nblk = NCORES * NB
    counts = np.bincount(blk_of, minlength=nblk)
    starts = np.zeros(nblk + 1, np.int64)
    np.cumsum(counts, out=starts[1:])
    tpb = int(np.ceil(max(1, counts.max()) / P))
    NT = NB * tpb

    # folds: w2k rows = [Wk ; ones-row (bk + edge-bias) ; Wea@We], per layer
    Wea = np.concatenate([W_edge, b_edge[None, :]], 0)        # [51, 64]
    mt = np.zeros((L, D + 1, H * XW), np.float32)
    wv2 = np.zeros((L, H, XW, D), np.float32)
    wska = np.zeros((L, D + 1, D), np.float32)
    scale = 1.0 / np.sqrt(D)
    for l in range(L):
        ew = Wea @ We[l]                                      # [51, 256]
        w2k = np.zeros((XW, H * D), np.float32)
        w2k[:D] = Wk[l]
        w2k[D] = ew[DE] + bk[l]
        w2k[D + 1:] = ew[:DE]
        Wq_aug = np.concatenate([Wq[l], bq[l][None, :]], 0)   # [65, 256]
        for h in range(H):
            mt[l, :, h * XW:(h + 1) * XW] = (
                Wq_aug[:, h * D:(h + 1) * D] @ w2k[:, h * D:(h + 1) * D].T) * scale
            wv2[l, h, :D] = Wv[l][:, h * D:(h + 1) * D] / H
            wv2[l, h, D] = (ew[DE, h * D:(h + 1) * D]
                            + bv[l][h * D:(h + 1) * D]) / H
            wv2[l, h, D + 1:] = ew[:DE, h * D:(h + 1) * D] / H
        wska[l, :D] = Wskip[l]
        wska[l, D] = bskip[l]
    watom = np.concatenate([W_atom, b_atom[None, :]], 0)
    wv2 = np.ascontiguousarray(np.transpose(wv2, (0, 2, 1, 3)).reshape(L, XW, H * D))

    # vectorized slab build: rank within dst-block -> (tile, partition) slot
    rank = np.arange(E, dtype=np.int64) - starts[blk_of]
    tt, pp = rank // P, rank % P
    eslab = np.zeros((NCORES, P, NB, tpb, SW), np.float32)
    eslab[:, :, :, :, 0] = 1.0
    mslab = np.full((NCORES, P, NB, tpb), -1.0, np.float32)
    islab = np.zeros((NCORES, P, NB, tpb), np.int32)
    cc, bb = blk_of // NB, blk_of % NB
    mslab[cc, pp, bb, tt] = (dst_s - blk_of * P).astype(np.float32)
    eslab[cc, pp, bb, tt, 1:] = ea_s
    islab[cc, pp, bb, tt] = src_s
    eslab = eslab.reshape(NCORES, P, NT * SW).astype(ml_dtypes.bfloat16)
    mslab = mslab.reshape(NCORES, P, NT)
    islab = islab.reshape(NCORES, P, NT)

    in_maps, g0s = [], []
    for c in range(NCORES):
        n0 = c * NLOC
        real = min(NLOC, max(0, N - n0))
        xaugT = np.zeros((DA + 1, NLOC), np.float32)
        xaugT[DA] = 1.0
        xaugT[:DA, :real] = x[n0:n0 + real].T
        brel = np.full((NLOC, 1), -1.0, np.float32)
        g0 = int(batch[min(n0, N - 1)]) if n0 < N else 0
        if real > 0:
            brel[:real, 0] = batch[n0:n0 + real] - g0
        g0s.append(g0)
        in_maps.append({
            "xaugT": xaugT,
            "ea_slab": eslab[c],
            "met_slab": mslab[c],
            "idx_slab": islab[c],
            "batch_rel": brel,
            "w_atom_aug": watom,
            "mt": mt.astype(ml_dtypes.bfloat16),
            "wv2": wv2.astype(ml_dtypes.bfloat16),
            "wska": wska.astype(ml_dtypes.bfloat16),
        })

    th.join()
    nc = _get_nc(tpb)
    res = run_bass_kernel_spmd(nc, in_maps, core_ids=list(range(NCORES)))

    sums = np.zeros((G + P, D), np.float64)
    cnts = np.zeros(G + P, np.float64)
    for c in range(NCORES):
        op = res.results[c]["out_pool"]
        sums[g0s[c]:g0s[c] + P] += op[:, :D]
        cnts[g0s[c]:g0s[c] + P] += op[:, D]
    pooled = sums[:G] / np.maximum(cnts[:G], 1.0)[:, None]
    out = pooled.astype(np.float32) @ W_out + b_out
    return out.squeeze()
